# revision 18
# baseline (speedup 1.0000x reference)
"""BiLSTM-CRF kernel for Trainium2 (8 NeuronCores, SPMD batch-sharded).

Device (Bass/Tile, 8 cores, 4 sequences each): the full emissions pipeline —
input projections (native fp16 matmul, biases folded in via a ones-row), both
LSTM recurrences (dynamic For_i over 64 chunks x 8 steps; gates in PSUM,
ACT-engine sigmoid/tanh, PE transposes keep h in contraction layout at f32r),
and the output projection. Host: embedding gather (shard prep) and the tiny
Viterbi decode.

All inputs ship as ONE packed fp16 blob per core (single device_put — the
axon link has a large fixed per-transfer cost); recurrence weights are cast
to f32r on device so the recurrence itself runs at the proven f32r accuracy.
All one-time work (axon/jax init, Bass build, walrus compile, PJRT load)
happens at module import; kernel() pays only host prep + transfer + execute.
"""

import os as _os
import sys
import time

for _p in ("/opt/trn_rl_repo", "/root/.axon_site/_ro/trn_rl_repo"):
    if _p not in sys.path:
        sys.path.insert(0, _p)

import numpy as np

B, L, V, E, H, T = 32, 512, 100000, 300, 256, 4
NCORES = 8
BPC = B // NCORES          # 4 sequences per core
TOK = BPC * L              # 2048
G4 = 4 * H                 # 1024
E_PAD = 384                # 300 data rows + ones row (bias) + zero pad
CHUNK = 8
BLOB_ROWS = E_PAD + E_PAD + H + 1   # xT | wihT | [whhT_f|whhT_b] | woutT-flat (all f32r rows)

LAST_DEVICE_NS = None      # device-portion wall time, read by test.py


# --------------------------------------------------------------------------
# Bass program: per-core emissions pipeline
# --------------------------------------------------------------------------
def _build_nc():
    import concourse.bacc as bacc
    import concourse.mybir as mybir
    from concourse.bass import ds
    from concourse.kernels.tile_matmul import matmul_tile_kernel
    from concourse.masks import make_identity
    from concourse.tile import TileContext

    F16 = mybir.dt.float16
    F32 = mybir.dt.float32
    F32R = mybir.dt.float32r
    AF = mybir.ActivationFunctionType

    nc = bacc.Bacc()
    blob = nc.declare_dram_parameter("blob", [BLOB_ROWS, 2 * G4], F32R, isOutput=False)
    emisT = nc.declare_dram_parameter("emisT", [T, TOK], F32, isOutput=True)

    xT = blob[0:E_PAD, :]
    wihT = blob[E_PAD : 2 * E_PAD, :]
    whh_v = {
        "f": blob[2 * E_PAD : 2 * E_PAD + H, 0:G4],
        "b": blob[2 * E_PAD : 2 * E_PAD + H, G4 : 2 * G4],
    }
    wout_v = blob[2 * E_PAD + H : 2 * E_PAD + H + 1, :].rearrange(
        "a (k n) -> (a k) n", n=T
    )  # [512, 4]

    if _os.environ.get("KERNEL_DEBUG"):
        xg = nc.declare_dram_parameter("xg_scratch", [TOK, 2 * G4], F32, isOutput=True)
        hsT = nc.declare_dram_parameter("hs_scratch", [2 * H, TOK], F32R, isOutput=True)
    else:
        xg = nc.dram_tensor("xg_scratch", [TOK, 2 * G4], F32, kind="Internal")
        hsT = nc.dram_tensor("hs_scratch", [2 * H, TOK], F32R, kind="Internal")

    # Phase A: xg[tok, 2048] = xT.T @ wihT  (native fp16; bias via ones-row)
    with TileContext(nc) as tc:
        matmul_tile_kernel(tc, xT, wihT, xg[:], matmul_dtype=F32R)

    xg_stg = xg.rearrange("(s t) g -> s t g", s=BPC)
    hsT_q = hsT.rearrange("(q p) (s t) -> q p s t", q=4, s=BPC)

    # Phase B: both LSTM recurrences (f32r compute)
    with TileContext(nc) as tc:
        with (
            tc.tile_pool(name="const", bufs=1) as const,
            tc.tile_pool(name="state", bufs=1) as state,
            tc.tile_pool(name="xgc", bufs=2) as xgp,
            tc.tile_pool(name="work", bufs=2) as work,
            tc.tile_pool(name="hsout", bufs=2) as hsp,
            tc.tile_pool(name="gps", bufs=1, space="PSUM") as gpsp,
            tc.tile_pool(name="trp", bufs=2, space="PSUM") as trp,
        ):
            identity = const.tile([128, 128], F32)
            make_identity(nc, identity[:])

            whh_sb = {}
            for d in "fb":
                t_ = const.tile([128, 2 * G4], F32R, tag=f"whh_{d}", name=f"whh_{d}")
                for k in range(2):
                    nc.sync.dma_start(
                        t_[:, k * G4 : (k + 1) * G4],
                        whh_v[d][k * 128 : (k + 1) * 128, :],
                    )
                whh_sb[d] = t_

            hT = {
                d: state.tile([128, 2 * BPC], F32R, tag=f"hT_{d}", name=f"hT_{d}")
                for d in "fb"
            }
            cst = {
                d: state.tile([BPC, H], F32, tag=f"c_{d}", name=f"c_{d}") for d in "fb"
            }
            zt = const.tile([128, 2 * BPC], F32, name="zt")
            nc.vector.memset(zt[:], 0.0)
            for d in "fb":
                nc.vector.tensor_copy(hT[d][:], zt[:])
                nc.vector.memset(cst[d][:], 0.0)

            with tc.For_i(0, L, CHUNK) as tok0:
                base_b = (L - CHUNK) - tok0
                xgc = {}
                for d, cb, col0 in (("f", tok0, 0), ("b", base_b, G4)):
                    t_ = xgp.tile(
                        [BPC, CHUNK * G4], F32, tag=f"xgc_{d}", name=f"xgc_{d}"
                    )
                    nc.sync.dma_start(
                        t_[:].rearrange("s (j g) -> s j g", j=CHUNK),
                        xg_stg[:, ds(cb, CHUNK), col0 : col0 + G4],
                    )
                    xgc[d] = t_

                hs_chunk = {
                    d: hsp.tile([128, CHUNK * 8], F32R, tag=f"hs_{d}", name=f"hs_{d}")
                    for d in "fb"
                }
                for rstep in range(CHUNK):
                    for d in "fb":
                        j = rstep if d == "f" else (CHUNK - 1) - rstep
                        g_ps = gpsp.tile([BPC, G4], F32, tag=f"g_{d}", name=f"g_{d}")
                        for n in range(2):
                            for k in range(2):
                                nc.tensor.matmul(
                                    g_ps[:, n * 512 : (n + 1) * 512],
                                    lhsT=hT[d][:, k * BPC : (k + 1) * BPC],
                                    rhs=whh_sb[d][
                                        :, k * G4 + n * 512 : k * G4 + (n + 1) * 512
                                    ],
                                    start=(k == 0),
                                    stop=(k == 1),
                                )
                        gsb = work.tile([BPC, G4], F32, tag=f"gsb_{d}", name=f"gsb_{d}")
                        nc.vector.tensor_add(
                            gsb[:], g_ps[:], xgc[d][:, j * G4 : (j + 1) * G4]
                        )
                        it_ = work.tile([BPC, H], F32, tag=f"i_{d}", name=f"i_{d}")
                        ft_ = work.tile([BPC, H], F32, tag=f"f_{d}", name=f"f_{d}")
                        gt_ = work.tile([BPC, H], F32, tag=f"g2_{d}", name=f"g2_{d}")
                        ot_ = work.tile([BPC, H], F32, tag=f"o_{d}", name=f"o_{d}")
                        nc.scalar.activation(it_[:], gsb[:, 0:H], AF.Sigmoid)
                        nc.scalar.activation(ft_[:], gsb[:, H : 2 * H], AF.Sigmoid)
                        nc.scalar.activation(gt_[:], gsb[:, 2 * H : 3 * H], AF.Tanh)
                        nc.scalar.activation(ot_[:], gsb[:, 3 * H : 4 * H], AF.Sigmoid)
                        t1 = work.tile([BPC, H], F32, tag=f"t1_{d}", name=f"t1_{d}")
                        nc.vector.tensor_mul(t1[:], ft_[:], cst[d][:])
                        t2 = work.tile([BPC, H], F32, tag=f"t2_{d}", name=f"t2_{d}")
                        nc.vector.tensor_mul(t2[:], it_[:], gt_[:])
                        nc.vector.tensor_add(cst[d][:], t1[:], t2[:])
                        th = work.tile([BPC, H], F32, tag=f"th_{d}", name=f"th_{d}")
                        nc.scalar.activation(th[:], cst[d][:], AF.Tanh)
                        ht_ = work.tile([BPC, H], F32, tag=f"h_{d}", name=f"h_{d}")
                        nc.vector.tensor_mul(ht_[:], ot_[:], th[:])
                        for half in range(2):
                            p_t = trp.tile([128, BPC], F32, tag="tr", name="tr")
                            nc.tensor.transpose(
                                p_t[:],
                                ht_[:, half * 128 : (half + 1) * 128],
                                identity[:BPC, :BPC],
                            )
                            nc.vector.tensor_copy(
                                hT[d][:, half * BPC : (half + 1) * BPC], p_t[:]
                            )
                            hs3 = hs_chunk[d][:].rearrange(
                                "p (s hh t) -> p s hh t", s=BPC, hh=2
                            )
                            nc.vector.tensor_copy(hs3[:, :, half, j].squeeze(), p_t[:])
                for d, cb in (("f", tok0), ("b", base_b)):
                    qbase = 0 if d == "f" else 2
                    src = hs_chunk[d][:].rearrange(
                        "p (s hh t) -> p hh s t", s=BPC, hh=2
                    )
                    for half in range(2):
                        nc.sync.dma_start(
                            hsT_q[qbase + half, :, :, ds(cb, CHUNK)].squeeze(),
                            src[:, half].squeeze(),
                        )

    # Phase C: emisT[4, 2048] = woutT.T @ hsT  (b_out added on host)
    with TileContext(nc) as tc:
        with (
            tc.tile_pool(name="hsb", bufs=1) as hsbp,
            tc.tile_pool(name="wout", bufs=1) as wop,
            tc.tile_pool(name="emis", bufs=1) as emp,
            tc.tile_pool(name="eps", bufs=2, space="PSUM") as epsp,
        ):
            hs_sb = hsbp.tile([128, 4 * TOK], F32R)
            for k in range(4):
                nc.sync.dma_start(
                    hs_sb[:, k * TOK : (k + 1) * TOK], hsT[k * 128 : (k + 1) * 128, :]
                )
            wo_sb = wop.tile([128, 4 * T], F32R, name="wo")
            for k in range(4):
                nc.sync.dma_start(
                    wo_sb[:, k * T : (k + 1) * T], wout_v[k * 128 : (k + 1) * 128, :]
                )
            em_sb = emp.tile([T, TOK], F32)
            for nchunk in range(4):
                n0 = nchunk * 512
                e_ps = epsp.tile([T, 512], F32, tag="eps", name="eps")
                for k in range(4):
                    nc.tensor.matmul(
                        e_ps[:],
                        lhsT=wo_sb[:, k * T : (k + 1) * T],
                        rhs=hs_sb[:, k * TOK + n0 : k * TOK + n0 + 512],
                        start=(k == 0),
                        stop=(k == 3),
                    )
                nc.vector.tensor_copy(em_sb[:, n0 : n0 + 512], e_ps[:])
            nc.sync.dma_start(emisT[:], em_sb[:])

    nc.finalize()
    return nc


# --------------------------------------------------------------------------
# PJRT runner: AOT-compiled shard_map over 8 cores (built at import)
# --------------------------------------------------------------------------
class _Runner:
    def __init__(self):
        import jax
        import jax.numpy as jnp
        from jax.experimental.shard_map import shard_map
        from jax.sharding import Mesh, NamedSharding, PartitionSpec

        import concourse.bass2jax as b2j
        import concourse.mybir as mybir

        self.jax = jax
        b2j.install_neuronx_cc_hook()

        nc = _build_nc()
        self.nc = nc

        in_names: list[str] = []
        out_names: list[str] = []
        out_avals = []
        partition_name = nc.partition_id_tensor.name if nc.partition_id_tensor else None
        for alloc in nc.m.functions[0].allocations:
            if not isinstance(alloc, mybir.MemoryLocationSet):
                continue
            name = alloc.memorylocations[0].name
            if alloc.kind == "ExternalInput":
                if name != partition_name:
                    in_names.append(name)
            elif alloc.kind == "ExternalOutput":
                out_names.append(name)
                shape = tuple(alloc.tensor_shape)
                dtype = mybir.dt.np(alloc.dtype)
                out_avals.append(jax.core.ShapedArray(shape, dtype))
        assert in_names == ["blob"], in_names
        self.out_names = list(out_names)
        all_names = in_names + out_names
        if partition_name is not None:
            all_names.append(partition_name)

        def _body(*args):
            operands = list(args)
            if partition_name is not None:
                operands.append(b2j.partition_id_tensor())
            outs = b2j._bass_exec_p.bind(
                *operands,
                out_avals=tuple(out_avals),
                in_names=tuple(all_names),
                out_names=tuple(out_names),
                lowering_input_output_aliases=(),
                sim_require_finite=True,
                sim_require_nnan=True,
                nc=nc,
            )
            return tuple(outs)

        devices = jax.devices()[:NCORES]
        mesh = Mesh(np.asarray(devices), ("core",))
        self.sharding = NamedSharding(mesh, PartitionSpec("core"))
        n_outs = len(out_names)
        jitted = jax.jit(
            shard_map(
                _body,
                mesh=mesh,
                in_specs=(PartitionSpec("core"),) * (1 + n_outs),
                out_specs=(PartitionSpec("core"),) * n_outs,
                check_rep=False,
            ),
            keep_unused=True,
        )
        sds = jax.ShapeDtypeStruct(
            (NCORES * BLOB_ROWS, 2 * G4), np.float32, sharding=self.sharding
        )
        sds_zeros = [
            jax.ShapeDtypeStruct(
                (NCORES * av.shape[0], *av.shape[1:]), av.dtype, sharding=self.sharding
            )
            for av in out_avals
        ]
        self.compiled = jitted.lower(sds, *sds_zeros).compile()
        # reusable zero output operands (kernel writes every output element,
        # and without donation these buffers are never consumed)
        self.zeros = [
            jax.device_put(
                np.zeros((NCORES * av.shape[0], *av.shape[1:]), av.dtype),
                self.sharding,
            )
            for av in out_avals
        ]
        # warm the h2d program, the executable, and the d2h path once
        dummy = np.zeros((NCORES * BLOB_ROWS, 2 * G4), np.float32)
        dummy_d = jax.device_put(dummy, self.sharding)
        warm = self.compiled(dummy_d, *self.zeros)
        np.asarray(warm[0])
        del dummy_d, warm

    def run(self, blob_all):
        jax = self.jax
        blob_d = jax.device_put(blob_all, self.sharding)
        outs = self.compiled(blob_d, *self.zeros)
        return {n: np.asarray(o) for n, o in zip(self.out_names, outs)}


import os as _os  # noqa: E402

_RUNNER = None if _os.environ.get("KERNEL_NO_INIT") else _Runner()


# --------------------------------------------------------------------------
# Host side
# --------------------------------------------------------------------------
def kernel(
    word_ids,
    mask,
    label_ids,
    emb,
    Wih_f,
    Whh_f,
    b_f,
    Wih_b,
    Whh_b,
    b_b,
    W_out,
    b_out,
    transitions,
    start_trans,
    end_trans,
):
    global LAST_DEVICE_NS, _RUNNER
    if _RUNNER is None:
        _RUNNER = _Runner()
    word_ids = np.asarray(word_ids, np.int32)
    mask = np.asarray(mask, np.int32)
    emb = np.asarray(emb, np.float32)
    W_out = np.asarray(W_out, np.float32)
    b_out = np.asarray(b_out, np.float32)

    # host prep: embedding gather + packed per-core fp32 blob
    x = emb[word_ids]  # [B, L, E] fp32
    blob_all = np.zeros((NCORES * BLOB_ROWS, 2 * G4), np.float32)

    wihT = np.zeros((E_PAD, 2 * G4), np.float32)
    wihT[:E, :G4] = np.asarray(Wih_f, np.float32).T
    wihT[:E, G4:] = np.asarray(Wih_b, np.float32).T
    wihT[E, :G4] = b_f
    wihT[E, G4:] = b_b
    whh_row = np.concatenate(
        [np.asarray(Whh_f, np.float32).T, np.asarray(Whh_b, np.float32).T], axis=1
    )  # [256, 2048]
    wout_flat = np.ascontiguousarray(W_out.T).reshape(1, 2 * G4)

    for c in range(NCORES):
        base = c * BLOB_ROWS
        xc = x[c * BPC : (c + 1) * BPC]  # [4, 512, 300]
        blob_all[base : base + E] = xc.transpose(2, 0, 1).reshape(E, TOK)
        blob_all[base + E] = 1.0
        blob_all[base + E_PAD : base + 2 * E_PAD] = wihT
        blob_all[base + 2 * E_PAD : base + 2 * E_PAD + H] = whh_row
        blob_all[base + 2 * E_PAD + H] = wout_flat

    t0 = time.perf_counter()
    outs = _RUNNER.run(blob_all)
    LAST_DEVICE_NS = int((time.perf_counter() - t0) * 1e9)

    emisT_all = outs["emisT"]  # [8*4, 2048]
    emissions = (
        emisT_all.reshape(NCORES, T, BPC, L).transpose(0, 2, 3, 1).reshape(B, L, T)
        + b_out
    )

    # Viterbi decode (host, mirrors reference exactly)
    trans = np.asarray(transitions, np.float32)
    m = mask.astype(bool)
    score = np.asarray(start_trans, np.float32) + emissions[:, 0]
    history = np.empty((L - 1, B, T), np.int32)
    for t in range(1, L):
        cand = score[:, :, None] + trans[None] + emissions[:, t][:, None, :]
        history[t - 1] = np.argmax(cand, axis=1).astype(np.int32)
        new = np.max(cand, axis=1)
        score = np.where(m[:, t][:, None], new, score)
    score = score + np.asarray(end_trans, np.float32)
    last_tag = np.argmax(score, axis=-1).astype(np.int32)

    tags = np.empty((B, L), np.int32)
    tags[:, L - 1] = last_tag
    tag = last_tag
    rows = np.arange(B)
    for t in range(L - 2, -1, -1):
        prev = history[t][rows, tag]
        tag = np.where(m[:, t + 1], prev, tag).astype(np.int32)
        tags[:, t] = tag
    return (tags * mask).astype(np.int32)


# revision 19
# speedup vs baseline: 1.0213x; 1.0213x over previous
"""BiLSTM-CRF kernel for Trainium2 (8 NeuronCores, SPMD batch-sharded).

Device (Bass/Tile, 8 cores, 4 sequences each): the full emissions pipeline —
input projections (f32r matmul, gate biases folded in via a ones-row of xT),
both LSTM recurrences (dynamic For_i over 64 chunks x 8 steps; gates in PSUM,
ACT-engine sigmoid/tanh, PE transposes keep h in the contraction layout), and
the output projection. Host: embedding gather (shard prep) and the tiny
Viterbi decode.

All inputs ship as ONE packed f32 blob per core (a single device_put — the
axon link has a large fixed per-transfer cost, so consolidating transfers
beats shrinking any single one). All one-time work (axon/jax init, Bass
build, walrus compile, PJRT load, transfer-path warmup) happens at module
import; kernel() pays only host prep + one transfer + execute + Viterbi.
"""

import os as _os
import sys
import time

for _p in ("/opt/trn_rl_repo", "/root/.axon_site/_ro/trn_rl_repo"):
    if _p not in sys.path:
        sys.path.insert(0, _p)

import numpy as np

B, L, V, E, H, T = 32, 512, 100000, 300, 256, 4
NCORES = 8
BPC = B // NCORES          # 4 sequences per core
TOK = BPC * L              # 2048
G4 = 4 * H                 # 1024
E_PAD = 384                # 300 data rows + ones row (bias) + zero pad
CHUNK = 8
BLOB_ROWS = E_PAD + E_PAD + H + 1   # xT | wihT | [whhT_f|whhT_b] | woutT-flat (all f32r rows)

LAST_DEVICE_NS = None      # device-portion wall time, read by test.py


# --------------------------------------------------------------------------
# Bass program: per-core emissions pipeline
# --------------------------------------------------------------------------
def _build_nc():
    import concourse.bacc as bacc
    import concourse.mybir as mybir
    from concourse.bass import ds
    from concourse.kernels.tile_matmul import matmul_tile_kernel
    from concourse.masks import make_identity
    from concourse.tile import TileContext

    F16 = mybir.dt.float16
    F32 = mybir.dt.float32
    F32R = mybir.dt.float32r
    AF = mybir.ActivationFunctionType

    nc = bacc.Bacc()
    blob = nc.declare_dram_parameter("blob", [BLOB_ROWS, 2 * G4], F32R, isOutput=False)
    emisT = nc.declare_dram_parameter("emisT", [T, TOK], F32, isOutput=True)

    xT = blob[0:E_PAD, :]
    wihT = blob[E_PAD : 2 * E_PAD, :]
    whh_v = {
        "f": blob[2 * E_PAD : 2 * E_PAD + H, 0:G4],
        "b": blob[2 * E_PAD : 2 * E_PAD + H, G4 : 2 * G4],
    }
    wout_v = blob[2 * E_PAD + H : 2 * E_PAD + H + 1, :].rearrange(
        "a (k n) -> (a k) n", n=T
    )  # [512, 4]

    if _os.environ.get("KERNEL_DEBUG"):
        xg = nc.declare_dram_parameter("xg_scratch", [TOK, 2 * G4], F32, isOutput=True)
        hsT = nc.declare_dram_parameter("hs_scratch", [2 * H, TOK], F32R, isOutput=True)
    else:
        xg = nc.dram_tensor("xg_scratch", [TOK, 2 * G4], F32, kind="Internal")
        hsT = nc.dram_tensor("hs_scratch", [2 * H, TOK], F32R, kind="Internal")

    # Phase A: xg[tok, 2048] = xT.T @ wihT  (f32r; bias via ones-row)
    with TileContext(nc) as tc:
        matmul_tile_kernel(tc, xT, wihT, xg[:], matmul_dtype=F32R)

    xg_stg = xg.rearrange("(s t) g -> s t g", s=BPC)
    hsT_q = hsT.rearrange("(q p) (s t) -> q p s t", q=4, s=BPC)

    # Phase B: both LSTM recurrences (f32r compute)
    with TileContext(nc) as tc:
        with (
            tc.tile_pool(name="const", bufs=1) as const,
            tc.tile_pool(name="state", bufs=1) as state,
            tc.tile_pool(name="xgc", bufs=2) as xgp,
            tc.tile_pool(name="work", bufs=2) as work,
            tc.tile_pool(name="hsout", bufs=2) as hsp,
            tc.tile_pool(name="gps", bufs=1, space="PSUM") as gpsp,
            tc.tile_pool(name="trp", bufs=2, space="PSUM") as trp,
        ):
            identity = const.tile([128, 128], F32)
            make_identity(nc, identity[:])

            whh_sb = {}
            for d in "fb":
                t_ = const.tile([128, 2 * G4], F32R, tag=f"whh_{d}", name=f"whh_{d}")
                for k in range(2):
                    nc.sync.dma_start(
                        t_[:, k * G4 : (k + 1) * G4],
                        whh_v[d][k * 128 : (k + 1) * 128, :],
                    )
                whh_sb[d] = t_

            hT = {
                d: state.tile([128, 2 * BPC], F32R, tag=f"hT_{d}", name=f"hT_{d}")
                for d in "fb"
            }
            cst = {
                d: state.tile([BPC, H], F32, tag=f"c_{d}", name=f"c_{d}") for d in "fb"
            }
            zt = const.tile([128, 2 * BPC], F32, name="zt")
            nc.vector.memset(zt[:], 0.0)
            for d in "fb":
                nc.vector.tensor_copy(hT[d][:], zt[:])
                nc.vector.memset(cst[d][:], 0.0)

            with tc.For_i(0, L, CHUNK) as tok0:
                base_b = (L - CHUNK) - tok0
                xgc = {}
                for d, cb, col0 in (("f", tok0, 0), ("b", base_b, G4)):
                    t_ = xgp.tile(
                        [BPC, CHUNK * G4], F32, tag=f"xgc_{d}", name=f"xgc_{d}"
                    )
                    nc.sync.dma_start(
                        t_[:].rearrange("s (j g) -> s j g", j=CHUNK),
                        xg_stg[:, ds(cb, CHUNK), col0 : col0 + G4],
                    )
                    xgc[d] = t_

                hs_chunk = {
                    d: hsp.tile([128, CHUNK * 8], F32R, tag=f"hs_{d}", name=f"hs_{d}")
                    for d in "fb"
                }
                for rstep in range(CHUNK):
                    for d in "fb":
                        j = rstep if d == "f" else (CHUNK - 1) - rstep
                        g_ps = gpsp.tile([BPC, G4], F32, tag=f"g_{d}", name=f"g_{d}")
                        for n in range(2):
                            for k in range(2):
                                nc.tensor.matmul(
                                    g_ps[:, n * 512 : (n + 1) * 512],
                                    lhsT=hT[d][:, k * BPC : (k + 1) * BPC],
                                    rhs=whh_sb[d][
                                        :, k * G4 + n * 512 : k * G4 + (n + 1) * 512
                                    ],
                                    start=(k == 0),
                                    stop=(k == 1),
                                )
                        gsb = work.tile([BPC, G4], F32, tag=f"gsb_{d}", name=f"gsb_{d}")
                        nc.vector.tensor_add(
                            gsb[:], g_ps[:], xgc[d][:, j * G4 : (j + 1) * G4]
                        )
                        it_ = work.tile([BPC, H], F32, tag=f"i_{d}", name=f"i_{d}")
                        ft_ = work.tile([BPC, H], F32, tag=f"f_{d}", name=f"f_{d}")
                        gt_ = work.tile([BPC, H], F32, tag=f"g2_{d}", name=f"g2_{d}")
                        ot_ = work.tile([BPC, H], F32, tag=f"o_{d}", name=f"o_{d}")
                        nc.scalar.activation(it_[:], gsb[:, 0:H], AF.Sigmoid)
                        nc.scalar.activation(ft_[:], gsb[:, H : 2 * H], AF.Sigmoid)
                        nc.scalar.activation(gt_[:], gsb[:, 2 * H : 3 * H], AF.Tanh)
                        nc.scalar.activation(ot_[:], gsb[:, 3 * H : 4 * H], AF.Sigmoid)
                        t1 = work.tile([BPC, H], F32, tag=f"t1_{d}", name=f"t1_{d}")
                        nc.vector.tensor_mul(t1[:], ft_[:], cst[d][:])
                        t2 = work.tile([BPC, H], F32, tag=f"t2_{d}", name=f"t2_{d}")
                        nc.vector.tensor_mul(t2[:], it_[:], gt_[:])
                        nc.vector.tensor_add(cst[d][:], t1[:], t2[:])
                        th = work.tile([BPC, H], F32, tag=f"th_{d}", name=f"th_{d}")
                        nc.scalar.activation(th[:], cst[d][:], AF.Tanh)
                        ht_ = work.tile([BPC, H], F32, tag=f"h_{d}", name=f"h_{d}")
                        nc.vector.tensor_mul(ht_[:], ot_[:], th[:])
                        for half in range(2):
                            p_t = trp.tile([128, BPC], F32, tag="tr", name="tr")
                            nc.tensor.transpose(
                                p_t[:],
                                ht_[:, half * 128 : (half + 1) * 128],
                                identity[:BPC, :BPC],
                            )
                            nc.vector.tensor_copy(
                                hT[d][:, half * BPC : (half + 1) * BPC], p_t[:]
                            )
                            hs3 = hs_chunk[d][:].rearrange(
                                "p (s hh t) -> p s hh t", s=BPC, hh=2
                            )
                            nc.vector.tensor_copy(hs3[:, :, half, j].squeeze(), p_t[:])
                for d, cb in (("f", tok0), ("b", base_b)):
                    qbase = 0 if d == "f" else 2
                    src = hs_chunk[d][:].rearrange(
                        "p (s hh t) -> p hh s t", s=BPC, hh=2
                    )
                    for half in range(2):
                        nc.sync.dma_start(
                            hsT_q[qbase + half, :, :, ds(cb, CHUNK)].squeeze(),
                            src[:, half].squeeze(),
                        )

    # Phase C: emisT[4, 2048] = woutT.T @ hsT  (b_out added on host)
    with TileContext(nc) as tc:
        with (
            tc.tile_pool(name="hsb", bufs=1) as hsbp,
            tc.tile_pool(name="wout", bufs=1) as wop,
            tc.tile_pool(name="emis", bufs=1) as emp,
            tc.tile_pool(name="eps", bufs=2, space="PSUM") as epsp,
        ):
            hs_sb = hsbp.tile([128, 4 * TOK], F32R)
            for k in range(4):
                nc.sync.dma_start(
                    hs_sb[:, k * TOK : (k + 1) * TOK], hsT[k * 128 : (k + 1) * 128, :]
                )
            wo_sb = wop.tile([128, 4 * T], F32R, name="wo")
            for k in range(4):
                nc.sync.dma_start(
                    wo_sb[:, k * T : (k + 1) * T], wout_v[k * 128 : (k + 1) * 128, :]
                )
            em_sb = emp.tile([T, TOK], F32)
            for nchunk in range(4):
                n0 = nchunk * 512
                e_ps = epsp.tile([T, 512], F32, tag="eps", name="eps")
                for k in range(4):
                    nc.tensor.matmul(
                        e_ps[:],
                        lhsT=wo_sb[:, k * T : (k + 1) * T],
                        rhs=hs_sb[:, k * TOK + n0 : k * TOK + n0 + 512],
                        start=(k == 0),
                        stop=(k == 3),
                    )
                nc.vector.tensor_copy(em_sb[:, n0 : n0 + 512], e_ps[:])
            nc.sync.dma_start(emisT[:], em_sb[:])

    nc.finalize()
    return nc


# --------------------------------------------------------------------------
# PJRT runner: AOT-compiled shard_map over 8 cores (built at import)
# --------------------------------------------------------------------------
class _Runner:
    def __init__(self):
        import jax
        import jax.numpy as jnp
        from jax.experimental.shard_map import shard_map
        from jax.sharding import Mesh, NamedSharding, PartitionSpec

        import concourse.bass2jax as b2j
        import concourse.mybir as mybir

        self.jax = jax
        b2j.install_neuronx_cc_hook()

        nc = _build_nc()
        self.nc = nc

        in_names: list[str] = []
        out_names: list[str] = []
        out_avals = []
        partition_name = nc.partition_id_tensor.name if nc.partition_id_tensor else None
        for alloc in nc.m.functions[0].allocations:
            if not isinstance(alloc, mybir.MemoryLocationSet):
                continue
            name = alloc.memorylocations[0].name
            if alloc.kind == "ExternalInput":
                if name != partition_name:
                    in_names.append(name)
            elif alloc.kind == "ExternalOutput":
                out_names.append(name)
                shape = tuple(alloc.tensor_shape)
                dtype = mybir.dt.np(alloc.dtype)
                out_avals.append(jax.core.ShapedArray(shape, dtype))
        assert in_names == ["blob"], in_names
        self.out_names = list(out_names)
        all_names = in_names + out_names
        if partition_name is not None:
            all_names.append(partition_name)

        def _body(*args):
            operands = list(args)
            if partition_name is not None:
                operands.append(b2j.partition_id_tensor())
            outs = b2j._bass_exec_p.bind(
                *operands,
                out_avals=tuple(out_avals),
                in_names=tuple(all_names),
                out_names=tuple(out_names),
                lowering_input_output_aliases=(),
                sim_require_finite=True,
                sim_require_nnan=True,
                nc=nc,
            )
            return tuple(outs)

        devices = jax.devices()[:NCORES]
        mesh = Mesh(np.asarray(devices), ("core",))
        self.sharding = NamedSharding(mesh, PartitionSpec("core"))
        n_outs = len(out_names)
        jitted = jax.jit(
            shard_map(
                _body,
                mesh=mesh,
                in_specs=(PartitionSpec("core"),) * (1 + n_outs),
                out_specs=(PartitionSpec("core"),) * n_outs,
                check_rep=False,
            ),
            keep_unused=True,
        )
        sds = jax.ShapeDtypeStruct(
            (NCORES * BLOB_ROWS, 2 * G4), np.float32, sharding=self.sharding
        )
        sds_zeros = [
            jax.ShapeDtypeStruct(
                (NCORES * av.shape[0], *av.shape[1:]), av.dtype, sharding=self.sharding
            )
            for av in out_avals
        ]
        self.compiled = jitted.lower(sds, *sds_zeros).compile()
        # reusable zero output operands (kernel writes every output element,
        # and without donation these buffers are never consumed)
        self.zeros = [
            jax.device_put(
                np.zeros((NCORES * av.shape[0], *av.shape[1:]), av.dtype),
                self.sharding,
            )
            for av in out_avals
        ]
        # warm the h2d program, the executable, and the d2h path once
        dummy = np.zeros((NCORES * BLOB_ROWS, 2 * G4), np.float32)
        dummy_d = jax.device_put(dummy, self.sharding)
        warm = self.compiled(dummy_d, *self.zeros)
        np.asarray(warm[0])
        del dummy_d, warm

    def run(self, blob_all):
        jax = self.jax
        blob_d = jax.device_put(blob_all, self.sharding)
        outs = self.compiled(blob_d, *self.zeros)
        return {n: np.asarray(o) for n, o in zip(self.out_names, outs)}


import os as _os  # noqa: E402

_RUNNER = None if _os.environ.get("KERNEL_NO_INIT") else _Runner()


# --------------------------------------------------------------------------
# Host side
# --------------------------------------------------------------------------
def kernel(
    word_ids,
    mask,
    label_ids,
    emb,
    Wih_f,
    Whh_f,
    b_f,
    Wih_b,
    Whh_b,
    b_b,
    W_out,
    b_out,
    transitions,
    start_trans,
    end_trans,
):
    global LAST_DEVICE_NS, _RUNNER
    if _RUNNER is None:
        _RUNNER = _Runner()
    word_ids = np.asarray(word_ids, np.int32)
    mask = np.asarray(mask, np.int32)
    emb = np.asarray(emb, np.float32)
    W_out = np.asarray(W_out, np.float32)
    b_out = np.asarray(b_out, np.float32)

    # host prep: embedding gather + packed per-core fp32 blob
    x = emb[word_ids]  # [B, L, E] fp32
    blob_all = np.zeros((NCORES * BLOB_ROWS, 2 * G4), np.float32)

    wihT = np.zeros((E_PAD, 2 * G4), np.float32)
    wihT[:E, :G4] = np.asarray(Wih_f, np.float32).T
    wihT[:E, G4:] = np.asarray(Wih_b, np.float32).T
    wihT[E, :G4] = b_f
    wihT[E, G4:] = b_b
    whh_row = np.concatenate(
        [np.asarray(Whh_f, np.float32).T, np.asarray(Whh_b, np.float32).T], axis=1
    )  # [256, 2048]
    wout_flat = np.ascontiguousarray(W_out.T).reshape(1, 2 * G4)

    for c in range(NCORES):
        base = c * BLOB_ROWS
        xc = x[c * BPC : (c + 1) * BPC]  # [4, 512, 300]
        blob_all[base : base + E] = xc.transpose(2, 0, 1).reshape(E, TOK)
        blob_all[base + E] = 1.0
        blob_all[base + E_PAD : base + 2 * E_PAD] = wihT
        blob_all[base + 2 * E_PAD : base + 2 * E_PAD + H] = whh_row
        blob_all[base + 2 * E_PAD + H] = wout_flat

    t0 = time.perf_counter()
    outs = _RUNNER.run(blob_all)
    LAST_DEVICE_NS = int((time.perf_counter() - t0) * 1e9)

    emisT_all = outs["emisT"]  # [8*4, 2048]
    emissions = (
        emisT_all.reshape(NCORES, T, BPC, L).transpose(0, 2, 3, 1).reshape(B, L, T)
        + b_out
    )

    # Viterbi decode (host, mirrors reference exactly)
    trans = np.asarray(transitions, np.float32)
    m = mask.astype(bool)
    score = np.asarray(start_trans, np.float32) + emissions[:, 0]
    history = np.empty((L - 1, B, T), np.int32)
    for t in range(1, L):
        cand = score[:, :, None] + trans[None] + emissions[:, t][:, None, :]
        history[t - 1] = np.argmax(cand, axis=1).astype(np.int32)
        new = np.max(cand, axis=1)
        score = np.where(m[:, t][:, None], new, score)
    score = score + np.asarray(end_trans, np.float32)
    last_tag = np.argmax(score, axis=-1).astype(np.int32)

    tags = np.empty((B, L), np.int32)
    tags[:, L - 1] = last_tag
    tag = last_tag
    rows = np.arange(B)
    for t in range(L - 2, -1, -1):
        prev = history[t][rows, tag]
        tag = np.where(m[:, t + 1], prev, tag).astype(np.int32)
        tags[:, t] = tag
    return (tags * mask).astype(np.int32)


# revision 20
# speedup vs baseline: 1.1499x; 1.1259x over previous
"""BiLSTM-CRF kernel for Trainium2 (8 NeuronCores, SPMD batch-sharded).

Device (Bass/Tile, 8 cores, 4 sequences each): the full emissions pipeline —
input projections (f32r matmul, gate biases folded in via a ones-row of xT),
both LSTM recurrences (dynamic For_i over 64 chunks x 8 steps; gates in PSUM,
ACT-engine sigmoid/tanh, PE transposes keep h in the contraction layout), and
the output projection. Host: embedding gather (shard prep) and the tiny
Viterbi decode.

All inputs ship as ONE packed f32 blob per core (a single device_put — the
axon link has a large fixed per-transfer cost, so consolidating transfers
beats shrinking any single one). All one-time work (axon/jax init, Bass
build, walrus compile, PJRT load, transfer-path warmup) happens at module
import; kernel() pays only host prep + one transfer + execute + Viterbi.
"""

import os as _os
import sys
import time

for _p in ("/opt/trn_rl_repo", "/root/.axon_site/_ro/trn_rl_repo"):
    if _p not in sys.path:
        sys.path.insert(0, _p)

import numpy as np

B, L, V, E, H, T = 32, 512, 100000, 300, 256, 4
NCORES = 8
BPC = B // NCORES          # 4 sequences per core
TOK = BPC * L              # 2048
G4 = 4 * H                 # 1024
E_PAD = 384                # 300 data rows + ones row (bias) + zero pad
CHUNK = 8
BLOB_ROWS = E_PAD + E_PAD + H + 1   # padded device layout: xT | wihT | whhT | woutT
SHIP_ROWS = (E + 1) + (E + 1) + H + 1  # 859 rows actually transferred (pad rebuilt on device)

LAST_DEVICE_NS = None      # device-portion wall time, read by test.py


# --------------------------------------------------------------------------
# Bass program: per-core emissions pipeline
# --------------------------------------------------------------------------
def _build_nc():
    import concourse.bacc as bacc
    import concourse.mybir as mybir
    from concourse.bass import ds
    from concourse.kernels.tile_matmul import matmul_tile_kernel
    from concourse.masks import make_identity
    from concourse.tile import TileContext

    F16 = mybir.dt.float16
    F32 = mybir.dt.float32
    F32R = mybir.dt.float32r
    AF = mybir.ActivationFunctionType

    nc = bacc.Bacc()
    ship = nc.declare_dram_parameter("ship", [SHIP_ROWS, 2 * G4], F32R, isOutput=False)
    emisT = nc.declare_dram_parameter("emisT", [T, TOK], F32, isOutput=True)

    # rebuild the zero-padded layout on device (save 16% of the transfer)
    blob = nc.dram_tensor("blob_pad", [BLOB_ROWS, 2 * G4], F32R, kind="Internal")
    with TileContext(nc) as tc:
        with tc.tile_pool(name="zp", bufs=1) as zp:
            z32 = zp.tile([128, TOK], F32, name="z32")
            nc.vector.memset(z32[:], 0.0)
            ztr = zp.tile([128, TOK], F32R, name="ztr")
            nc.vector.tensor_copy(ztr[:], z32[:])
            nc.sync.dma_start(blob[0 : E + 1, :], ship[0 : E + 1, :])
            nc.sync.dma_start(blob[E + 1 : E_PAD, :], ztr[: E_PAD - E - 1, :])
            nc.sync.dma_start(
                blob[E_PAD : E_PAD + E + 1, :], ship[E + 1 : 2 * (E + 1), :]
            )
            nc.sync.dma_start(
                blob[E_PAD + E + 1 : 2 * E_PAD, :], ztr[: E_PAD - E - 1, :]
            )
            nc.sync.dma_start(
                blob[2 * E_PAD : BLOB_ROWS, :], ship[2 * (E + 1) : SHIP_ROWS, :]
            )

    xT = blob[0:E_PAD, :]
    wihT = blob[E_PAD : 2 * E_PAD, :]
    whh_v = {
        "f": blob[2 * E_PAD : 2 * E_PAD + H, 0:G4],
        "b": blob[2 * E_PAD : 2 * E_PAD + H, G4 : 2 * G4],
    }
    wout_v = blob[2 * E_PAD + H : 2 * E_PAD + H + 1, :].rearrange(
        "a (k n) -> (a k) n", n=T
    )  # [512, 4]

    if _os.environ.get("KERNEL_DEBUG"):
        xg = nc.declare_dram_parameter("xg_scratch", [TOK, 2 * G4], F32, isOutput=True)
        hsT = nc.declare_dram_parameter("hs_scratch", [2 * H, TOK], F32R, isOutput=True)
    else:
        xg = nc.dram_tensor("xg_scratch", [TOK, 2 * G4], F32, kind="Internal")
        hsT = nc.dram_tensor("hs_scratch", [2 * H, TOK], F32R, kind="Internal")

    # Phase A: xg[tok, 2048] = xT.T @ wihT  (f32r; bias via ones-row)
    with TileContext(nc) as tc:
        matmul_tile_kernel(tc, xT, wihT, xg[:], matmul_dtype=F32R)

    xg_stg = xg.rearrange("(s t) g -> s t g", s=BPC)
    hsT_q = hsT.rearrange("(q p) (s t) -> q p s t", q=4, s=BPC)

    # Phase B: both LSTM recurrences (f32r compute)
    with TileContext(nc) as tc:
        with (
            tc.tile_pool(name="const", bufs=1) as const,
            tc.tile_pool(name="state", bufs=1) as state,
            tc.tile_pool(name="xgc", bufs=2) as xgp,
            tc.tile_pool(name="work", bufs=2) as work,
            tc.tile_pool(name="hsout", bufs=2) as hsp,
            tc.tile_pool(name="gps", bufs=1, space="PSUM") as gpsp,
            tc.tile_pool(name="trp", bufs=2, space="PSUM") as trp,
        ):
            identity = const.tile([128, 128], F32)
            make_identity(nc, identity[:])

            whh_sb = {}
            for d in "fb":
                t_ = const.tile([128, 2 * G4], F32R, tag=f"whh_{d}", name=f"whh_{d}")
                for k in range(2):
                    nc.sync.dma_start(
                        t_[:, k * G4 : (k + 1) * G4],
                        whh_v[d][k * 128 : (k + 1) * 128, :],
                    )
                whh_sb[d] = t_

            hT = {
                d: state.tile([128, 2 * BPC], F32R, tag=f"hT_{d}", name=f"hT_{d}")
                for d in "fb"
            }
            cst = {
                d: state.tile([BPC, H], F32, tag=f"c_{d}", name=f"c_{d}") for d in "fb"
            }
            zt = const.tile([128, 2 * BPC], F32, name="zt")
            nc.vector.memset(zt[:], 0.0)
            for d in "fb":
                nc.vector.tensor_copy(hT[d][:], zt[:])
                nc.vector.memset(cst[d][:], 0.0)

            with tc.For_i(0, L, CHUNK) as tok0:
                base_b = (L - CHUNK) - tok0
                xgc = {}
                for d, cb, col0 in (("f", tok0, 0), ("b", base_b, G4)):
                    t_ = xgp.tile(
                        [BPC, CHUNK * G4], F32, tag=f"xgc_{d}", name=f"xgc_{d}"
                    )
                    nc.sync.dma_start(
                        t_[:].rearrange("s (j g) -> s j g", j=CHUNK),
                        xg_stg[:, ds(cb, CHUNK), col0 : col0 + G4],
                    )
                    xgc[d] = t_

                hs_chunk = {
                    d: hsp.tile([128, CHUNK * 8], F32R, tag=f"hs_{d}", name=f"hs_{d}")
                    for d in "fb"
                }
                for rstep in range(CHUNK):
                    for d in "fb":
                        j = rstep if d == "f" else (CHUNK - 1) - rstep
                        g_ps = gpsp.tile([BPC, G4], F32, tag=f"g_{d}", name=f"g_{d}")
                        for n in range(2):
                            for k in range(2):
                                nc.tensor.matmul(
                                    g_ps[:, n * 512 : (n + 1) * 512],
                                    lhsT=hT[d][:, k * BPC : (k + 1) * BPC],
                                    rhs=whh_sb[d][
                                        :, k * G4 + n * 512 : k * G4 + (n + 1) * 512
                                    ],
                                    start=(k == 0),
                                    stop=(k == 1),
                                )
                        gsb = work.tile([BPC, G4], F32, tag=f"gsb_{d}", name=f"gsb_{d}")
                        nc.vector.tensor_add(
                            gsb[:], g_ps[:], xgc[d][:, j * G4 : (j + 1) * G4]
                        )
                        it_ = work.tile([BPC, H], F32, tag=f"i_{d}", name=f"i_{d}")
                        ft_ = work.tile([BPC, H], F32, tag=f"f_{d}", name=f"f_{d}")
                        gt_ = work.tile([BPC, H], F32, tag=f"g2_{d}", name=f"g2_{d}")
                        ot_ = work.tile([BPC, H], F32, tag=f"o_{d}", name=f"o_{d}")
                        nc.scalar.activation(it_[:], gsb[:, 0:H], AF.Sigmoid)
                        nc.scalar.activation(ft_[:], gsb[:, H : 2 * H], AF.Sigmoid)
                        nc.scalar.activation(gt_[:], gsb[:, 2 * H : 3 * H], AF.Tanh)
                        nc.scalar.activation(ot_[:], gsb[:, 3 * H : 4 * H], AF.Sigmoid)
                        t1 = work.tile([BPC, H], F32, tag=f"t1_{d}", name=f"t1_{d}")
                        nc.vector.tensor_mul(t1[:], ft_[:], cst[d][:])
                        t2 = work.tile([BPC, H], F32, tag=f"t2_{d}", name=f"t2_{d}")
                        nc.vector.tensor_mul(t2[:], it_[:], gt_[:])
                        nc.vector.tensor_add(cst[d][:], t1[:], t2[:])
                        th = work.tile([BPC, H], F32, tag=f"th_{d}", name=f"th_{d}")
                        nc.scalar.activation(th[:], cst[d][:], AF.Tanh)
                        ht_ = work.tile([BPC, H], F32, tag=f"h_{d}", name=f"h_{d}")
                        nc.vector.tensor_mul(ht_[:], ot_[:], th[:])
                        for half in range(2):
                            p_t = trp.tile([128, BPC], F32, tag="tr", name="tr")
                            nc.tensor.transpose(
                                p_t[:],
                                ht_[:, half * 128 : (half + 1) * 128],
                                identity[:BPC, :BPC],
                            )
                            nc.vector.tensor_copy(
                                hT[d][:, half * BPC : (half + 1) * BPC], p_t[:]
                            )
                            hs3 = hs_chunk[d][:].rearrange(
                                "p (s hh t) -> p s hh t", s=BPC, hh=2
                            )
                            nc.vector.tensor_copy(hs3[:, :, half, j].squeeze(), p_t[:])
                for d, cb in (("f", tok0), ("b", base_b)):
                    qbase = 0 if d == "f" else 2
                    src = hs_chunk[d][:].rearrange(
                        "p (s hh t) -> p hh s t", s=BPC, hh=2
                    )
                    for half in range(2):
                        nc.sync.dma_start(
                            hsT_q[qbase + half, :, :, ds(cb, CHUNK)].squeeze(),
                            src[:, half].squeeze(),
                        )

    # Phase C: emisT[4, 2048] = woutT.T @ hsT  (b_out added on host)
    with TileContext(nc) as tc:
        with (
            tc.tile_pool(name="hsb", bufs=1) as hsbp,
            tc.tile_pool(name="wout", bufs=1) as wop,
            tc.tile_pool(name="emis", bufs=1) as emp,
            tc.tile_pool(name="eps", bufs=2, space="PSUM") as epsp,
        ):
            hs_sb = hsbp.tile([128, 4 * TOK], F32R)
            for k in range(4):
                nc.sync.dma_start(
                    hs_sb[:, k * TOK : (k + 1) * TOK], hsT[k * 128 : (k + 1) * 128, :]
                )
            wo_sb = wop.tile([128, 4 * T], F32R, name="wo")
            for k in range(4):
                nc.sync.dma_start(
                    wo_sb[:, k * T : (k + 1) * T], wout_v[k * 128 : (k + 1) * 128, :]
                )
            em_sb = emp.tile([T, TOK], F32)
            for nchunk in range(4):
                n0 = nchunk * 512
                e_ps = epsp.tile([T, 512], F32, tag="eps", name="eps")
                for k in range(4):
                    nc.tensor.matmul(
                        e_ps[:],
                        lhsT=wo_sb[:, k * T : (k + 1) * T],
                        rhs=hs_sb[:, k * TOK + n0 : k * TOK + n0 + 512],
                        start=(k == 0),
                        stop=(k == 3),
                    )
                nc.vector.tensor_copy(em_sb[:, n0 : n0 + 512], e_ps[:])
            nc.sync.dma_start(emisT[:], em_sb[:])

    nc.finalize()
    return nc


# --------------------------------------------------------------------------
# PJRT runner: AOT-compiled shard_map over 8 cores (built at import)
# --------------------------------------------------------------------------
class _Runner:
    def __init__(self):
        import jax
        import jax.numpy as jnp
        from jax.experimental.shard_map import shard_map
        from jax.sharding import Mesh, NamedSharding, PartitionSpec

        import concourse.bass2jax as b2j
        import concourse.mybir as mybir

        self.jax = jax
        b2j.install_neuronx_cc_hook()

        nc = _build_nc()
        self.nc = nc

        in_names: list[str] = []
        out_names: list[str] = []
        out_avals = []
        partition_name = nc.partition_id_tensor.name if nc.partition_id_tensor else None
        for alloc in nc.m.functions[0].allocations:
            if not isinstance(alloc, mybir.MemoryLocationSet):
                continue
            name = alloc.memorylocations[0].name
            if alloc.kind == "ExternalInput":
                if name != partition_name:
                    in_names.append(name)
            elif alloc.kind == "ExternalOutput":
                out_names.append(name)
                shape = tuple(alloc.tensor_shape)
                dtype = mybir.dt.np(alloc.dtype)
                out_avals.append(jax.core.ShapedArray(shape, dtype))
        assert in_names == ["ship"], in_names
        self.out_names = list(out_names)
        all_names = in_names + out_names
        if partition_name is not None:
            all_names.append(partition_name)

        def _body(*args):
            operands = list(args)
            if partition_name is not None:
                operands.append(b2j.partition_id_tensor())
            outs = b2j._bass_exec_p.bind(
                *operands,
                out_avals=tuple(out_avals),
                in_names=tuple(all_names),
                out_names=tuple(out_names),
                lowering_input_output_aliases=(),
                sim_require_finite=True,
                sim_require_nnan=True,
                nc=nc,
            )
            return tuple(outs)

        devices = jax.devices()[:NCORES]
        mesh = Mesh(np.asarray(devices), ("core",))
        self.sharding = NamedSharding(mesh, PartitionSpec("core"))
        n_outs = len(out_names)
        jitted = jax.jit(
            shard_map(
                _body,
                mesh=mesh,
                in_specs=(PartitionSpec("core"),) * (1 + n_outs),
                out_specs=(PartitionSpec("core"),) * n_outs,
                check_rep=False,
            ),
            keep_unused=True,
        )
        sds = jax.ShapeDtypeStruct(
            (NCORES * SHIP_ROWS, 2 * G4), np.float32, sharding=self.sharding
        )
        sds_zeros = [
            jax.ShapeDtypeStruct(
                (NCORES * av.shape[0], *av.shape[1:]), av.dtype, sharding=self.sharding
            )
            for av in out_avals
        ]
        self.compiled = jitted.lower(sds, *sds_zeros).compile()
        # reusable zero output operands (kernel writes every output element,
        # and without donation these buffers are never consumed)
        self.zeros = [
            jax.device_put(
                np.zeros((NCORES * av.shape[0], *av.shape[1:]), av.dtype),
                self.sharding,
            )
            for av in out_avals
        ]
        # warm the h2d program, the executable, and the d2h path once
        dummy = np.zeros((NCORES * SHIP_ROWS, 2 * G4), np.float32)
        dummy_d = jax.device_put(dummy, self.sharding)
        warm = self.compiled(dummy_d, *self.zeros)
        np.asarray(warm[0])
        del dummy_d, warm

    def run(self, blob_all):
        jax = self.jax
        blob_d = jax.device_put(blob_all, self.sharding)
        outs = self.compiled(blob_d, *self.zeros)
        return {n: np.asarray(o) for n, o in zip(self.out_names, outs)}


import os as _os  # noqa: E402

_RUNNER = None if _os.environ.get("KERNEL_NO_INIT") else _Runner()


# --------------------------------------------------------------------------
# Host side
# --------------------------------------------------------------------------
def kernel(
    word_ids,
    mask,
    label_ids,
    emb,
    Wih_f,
    Whh_f,
    b_f,
    Wih_b,
    Whh_b,
    b_b,
    W_out,
    b_out,
    transitions,
    start_trans,
    end_trans,
):
    global LAST_DEVICE_NS, _RUNNER
    if _RUNNER is None:
        _RUNNER = _Runner()
    word_ids = np.asarray(word_ids, np.int32)
    mask = np.asarray(mask, np.int32)
    emb = np.asarray(emb, np.float32)
    W_out = np.asarray(W_out, np.float32)
    b_out = np.asarray(b_out, np.float32)

    # host prep: embedding gather + packed per-core fp32 blob (no pad rows)
    x = emb[word_ids]  # [B, L, E] fp32
    blob_all = np.empty((NCORES * SHIP_ROWS, 2 * G4), np.float32)

    wih_rows = np.empty((E + 1, 2 * G4), np.float32)
    wih_rows[:E, :G4] = np.asarray(Wih_f, np.float32).T
    wih_rows[:E, G4:] = np.asarray(Wih_b, np.float32).T
    wih_rows[E, :G4] = b_f
    wih_rows[E, G4:] = b_b
    whh_row = np.concatenate(
        [np.asarray(Whh_f, np.float32).T, np.asarray(Whh_b, np.float32).T], axis=1
    )  # [256, 2048]
    wout_flat = np.ascontiguousarray(W_out.T).reshape(1, 2 * G4)

    for c in range(NCORES):
        base = c * SHIP_ROWS
        xc = x[c * BPC : (c + 1) * BPC]  # [4, 512, 300]
        blob_all[base : base + E] = xc.transpose(2, 0, 1).reshape(E, TOK)
        blob_all[base + E] = 1.0
        blob_all[base + E + 1 : base + 2 * (E + 1)] = wih_rows
        blob_all[base + 2 * (E + 1) : base + 2 * (E + 1) + H] = whh_row
        blob_all[base + 2 * (E + 1) + H] = wout_flat

    t0 = time.perf_counter()
    outs = _RUNNER.run(blob_all)
    LAST_DEVICE_NS = int((time.perf_counter() - t0) * 1e9)

    emisT_all = outs["emisT"]  # [8*4, 2048]
    emissions = (
        emisT_all.reshape(NCORES, T, BPC, L).transpose(0, 2, 3, 1).reshape(B, L, T)
        + b_out
    )

    # Viterbi decode (host, mirrors reference exactly)
    trans = np.asarray(transitions, np.float32)
    m = mask.astype(bool)
    score = np.asarray(start_trans, np.float32) + emissions[:, 0]
    history = np.empty((L - 1, B, T), np.int32)
    for t in range(1, L):
        cand = score[:, :, None] + trans[None] + emissions[:, t][:, None, :]
        history[t - 1] = np.argmax(cand, axis=1).astype(np.int32)
        new = np.max(cand, axis=1)
        score = np.where(m[:, t][:, None], new, score)
    score = score + np.asarray(end_trans, np.float32)
    last_tag = np.argmax(score, axis=-1).astype(np.int32)

    tags = np.empty((B, L), np.int32)
    tags[:, L - 1] = last_tag
    tag = last_tag
    rows = np.arange(B)
    for t in range(L - 2, -1, -1):
        prev = history[t][rows, tag]
        tag = np.where(m[:, t + 1], prev, tag).astype(np.int32)
        tags[:, t] = tag
    return (tags * mask).astype(np.int32)
